# revision 2
# baseline (speedup 1.0000x reference)
"""Trainium2 Bass kernel v2 for nn_MultiHeadAttention (B=4, S=2048, D=1024, H=16).

Sharding: core c handles batch c//2 and heads 8*(c%2)..+8 (data-parallel over
batch x Megatron head-parallel; per-core partial outputs summed pairwise on
host, which also adds b_o).

v2 vs baseline:
- bf16 operands everywhere (halves DMA + SBUF, enables DVE 4x modes); psum
  accumulation stays fp32, denominators/reciprocals fp32.
- fused window pipeline: project K/V/Q for seq window w, then attention for
  q-chunk w over kt<=4w+3 -- ACT exp overlaps projection matmuls.
- head-pair score matmuls (K=64) issued back-to-back at base partitions 0/64:
  auto tile_position row groups {0,1}/{2,3} -> concurrent on the PE array.
- rope permutation via bf16 PE matmul, mults/adds on DVE in 4x mode.
- bias matmuls elided when biases are zero (variant-cached build).
"""

import numpy as np
import ml_dtypes
import concourse.bass as bass
import concourse.tile as tile
from concourse import mybir, bacc

F32 = mybir.dt.float32
BF = mybir.dt.bfloat16
AF = mybir.ActivationFunctionType
ALU = mybir.AluOpType

B, S, D, H = 4, 2048, 1024, 16
DK = D // H          # 64
HC = 8               # heads per core
HD = HC * DK         # 512 head dims per core
KT = S // 128        # 16 key tiles
NWIN = 4             # seq windows of 512
NMT = HD // 128      # 4 M-tiles (head-pair groups g)
DMT = D // 128       # 8 M-tiles for output projection
VB = HD + HC         # 520 vaug cols per kt block

bf16 = ml_dtypes.bfloat16


def build_nc(reps=1, timing=False, phases=(1, 1), with_bias=(False, False, False),
             fuse=True, exp3d=True):
    nc = bacc.Bacc(None, target_bir_lowering=False)
    bias_q, bias_k, bias_v = with_bias

    if timing:
        def declare(name, shape, dtype, isOutput=False):
            return nc.dram_tensor(name, shape, dtype)
        dummy_in = nc.declare_dram_parameter("dummy_in", [128, 128], F32,
                                             isOutput=False)
        dummy_out = nc.declare_dram_parameter("dummy_out", [128, 128], F32,
                                              isOutput=True)
    else:
        declare = nc.declare_dram_parameter

    xt_q = declare("xt_q", [D, S], BF, isOutput=False)
    xt_k = declare("xt_k", [D, S], BF, isOutput=False)
    xt_v = declare("xt_v", [D, S], BF, isOutput=False)
    wqt = declare("wqt", [D, HD], BF, isOutput=False)
    wkt = declare("wkt", [D, HD], BF, isOutput=False)
    wvt = declare("wvt", [D, HD], BF, isOutput=False)
    wot = declare("wot", [128, NMT * D], BF, isOutput=False)
    pmat = declare("pmat", [128, 128], BF, isOutput=False)
    cost = declare("cost", [128, S], BF, isOutput=False)
    sint = declare("sint", [128, S], BF, isOutput=False)
    trim = declare("trim", [128, 128], BF, isOutput=False)
    if bias_q:
        bqp = declare("bqp", [128, NMT], F32, isOutput=False)
    if bias_k:
        bkp = declare("bkp", [128, NMT], F32, isOutput=False)
    if bias_v:
        bvp = declare("bvp", [1, HD], BF, isOutput=False)
    out_pt = declare("out_pt", [D, S], BF, isOutput=True)

    with tile.TileContext(nc) as tc:
      if timing:
          with tc.tile_pool(name="dummy", bufs=1) as dp:
              dt_ = dp.tile([128, 128], F32, tag="dt_")
              nc.sync.dma_start(dt_[:], dummy_in[:])
              nc.sync.dma_start(dummy_out[:], dt_[:])
      for _rep in range(reps):
        with tc.tile_pool(name="pers", bufs=1) as pers:
            # ---------------- persistent tiles + upfront loads ----------------
            # K-projection weights + rope tables first: window-0 K-proj needs
            # them immediately; V/Q/O weights stream in behind window-0 x data.
            wq_sb = pers.tile([128, 8 * HD], BF, tag="wq_sb")
            wk_sb = pers.tile([128, 8 * HD], BF, tag="wk_sb")
            wv_sb = pers.tile([128, 8 * HD], BF, tag="wv_sb")
            for k in range(8):
                nc.sync.dma_start(wk_sb[:, k * HD:(k + 1) * HD],
                                  wkt[k * 128:(k + 1) * 128, :])
            cos_sb = pers.tile([128, S], BF, tag="cos_sb")
            nc.sync.dma_start(cos_sb[:], cost[:])
            sin_sb = pers.tile([128, S], BF, tag="sin_sb")
            nc.sync.dma_start(sin_sb[:], sint[:])
            pm_sb = pers.tile([128, 128], BF, tag="pm_sb")
            nc.sync.dma_start(pm_sb[:], pmat[:])
            wo_sb = pers.tile([128, NMT * D], BF, tag="wo_sb")
            tri = pers.tile([128, 128], BF, tag="tri")
            nc.sync.dma_start(tri[:], trim[:])
            ones_b = pers.tile([128, 128], BF, tag="ones_b")
            nc.vector.memset(ones_b[:], 1.0)
            if bias_q:
                bq_sb = pers.tile([128, NMT], F32, tag="bq_sb")
                nc.sync.dma_start(bq_sb[:], bqp[:])
            if bias_k:
                bk_sb = pers.tile([128, NMT], F32, tag="bk_sb")
                nc.sync.dma_start(bk_sb[:], bkp[:])
            if bias_v:
                bv_sb = pers.tile([1, HD], BF, tag="bv_sb")
                nc.sync.dma_start(bv_sb[:], bvp[:])

            qrt = pers.tile([128, NMT * S], BF, tag="qrt")
            krt = pers.tile([128, NMT * S], BF, tag="krt")
            vaug = pers.tile([128, KT * VB], BF, tag="vaug")
            ctxt = pers.tile([128, NMT * S], BF, tag="ctxt")
            # ones columns of vaug (col 64 of each 65-col head block)
            nc.vector.memset(
                vaug[:].rearrange("p (g c) -> p g c", c=DK + 1)[:, :, DK:DK + 1],
                1.0)

            with tc.tile_pool(name="xk", bufs=2) as xkp, \
                 tc.tile_pool(name="xv", bufs=2) as xvp, \
                 tc.tile_pool(name="xq", bufs=2) as xqp, \
                 tc.tile_pool(name="qts", bufs=3) as qts, \
                 tc.tile_pool(name="rts", bufs=4) as rts, \
                 tc.tile_pool(name="exps", bufs=5) as exps, \
                 tc.tile_pool(name="dens", bufs=2) as dens, \
                 tc.tile_pool(name="bcs", bufs=2) as bcs, \
                 tc.tile_pool(name="stg", bufs=2) as stg, \
                 tc.tile_pool(name="outs", bufs=3) as outs, \
                 tc.tile_pool(name="psS", bufs=2, space="PSUM") as psS, \
                 tc.tile_pool(name="psC", bufs=1, space="PSUM") as psC, \
                 tc.tile_pool(name="psP", bufs=2, space="PSUM") as psP:

                def proj_qk(xt, w_sb, b_sb, dst, w):
                    """Project+rope window w of xt into dst (qrt/krt)."""
                    xq = xqp.tile([128, 8 * 512], BF, tag="xq")
                    for k in range(8):
                        nc.sync.dma_start(
                            xq[:, k * 512:(k + 1) * 512],
                            xt[k * 128:(k + 1) * 128, w * 512:(w + 1) * 512])
                    for mt in range(NMT):
                        psq = psP.tile([128, 512], F32, tag="psq")
                        for k in range(8):
                            nc.tensor.matmul(
                                psq[:],
                                w_sb[:, k * HD + 128 * mt: k * HD + 128 * mt + 128],
                                xq[:, k * 512:(k + 1) * 512],
                                start=(k == 0), stop=(k == 7))
                        qt = qts.tile([128, 512], BF, tag="qt")
                        if b_sb is not None:
                            nc.scalar.activation(qt[:], psq[:], AF.Identity,
                                                 bias=b_sb[:, mt:mt + 1], scale=1.0)
                        else:
                            nc.vector.tensor_copy(qt[:], psq[:])
                        psp = psP.tile([128, 512], F32, tag="psq")
                        nc.tensor.matmul(psp[:], pm_sb[:], qt[:],
                                         start=True, stop=True)
                        cw = cos_sb[:, w * 512:(w + 1) * 512]
                        sw = sin_sb[:, w * 512:(w + 1) * 512]
                        t1 = rts.tile([128, 512], BF, tag="t1")
                        nc.vector.tensor_tensor(t1[:], qt[:], cw, ALU.mult)
                        t2 = rts.tile([128, 512], BF, tag="t2")
                        nc.vector.tensor_tensor(t2[:], psp[:], sw, ALU.mult)
                        nc.vector.tensor_add(
                            dst[:, mt * S + w * 512: mt * S + (w + 1) * 512],
                            t1[:], t2[:])

                def proj_v(w):
                    if w == 0:
                        for k in range(8):
                            nc.sync.dma_start(wv_sb[:, k * HD:(k + 1) * HD],
                                              wvt[k * 128:(k + 1) * 128, :])
                    xv = xvp.tile([128, 8 * 512], BF, tag="xv")
                    for k in range(8):
                        nc.sync.dma_start(
                            xv[:, k * 512:(k + 1) * 512],
                            xt_v[k * 128:(k + 1) * 128, w * 512:(w + 1) * 512])
                    for st in range(4):
                        psv = psP.tile([128, 512], F32, tag="psq")
                        for k in range(8):
                            nc.tensor.matmul(
                                psv[:],
                                xv[:, k * 512 + st * 128: k * 512 + st * 128 + 128],
                                wv_sb[:, k * HD:(k + 1) * HD],
                                start=(k == 0), stop=(k == 7 and not bias_v))
                        if bias_v:
                            nc.tensor.matmul(psv[:], ones_b[0:1, 0:128], bv_sb[:],
                                             start=False, stop=True)
                        kt = w * 4 + st
                        nc.vector.tensor_copy(
                            vaug[:, kt * VB: kt * VB + VB].rearrange(
                                "p (h c) -> p h c", c=DK + 1)[:, :, 0:DK],
                            psv[:].rearrange("p (h c) -> p h c", c=DK))

                def attend(w):
                    """Attention for q-chunk w (cols qs..qs+512) + out proj."""
                    qs = w * 512
                    for g in range(NMT):
                        hA, hB = 2 * g, 2 * g + 1
                        psc = psC.tile([65, 1024], F32, tag="psc")
                        ktmax = 4 * w + 3
                        # kt ascending: ctx matmul for kt=0 is full-width with
                        # start=True (clears + has_written for the whole bank),
                        # so diagonal kts accumulate at exact partial width --
                        # no below-diagonal zero-fill needed.
                        for kt in range(0, ktmax + 1):
                            q0 = max(128 * kt, qs)
                            lo = q0 - qs                      # 0..383 within chunk
                            pss = psS.tile([128, 1024], F32, tag="pss")
                            nc.tensor.matmul(
                                pss[:, lo:512],
                                krt[0:64, g * S + kt * 128: g * S + kt * 128 + 128],
                                qrt[0:64, g * S + q0: g * S + qs + 512],
                                start=True, stop=True)
                            nc.tensor.matmul(
                                pss[:, 512 + lo:1024],
                                krt[64:128, g * S + kt * 128: g * S + kt * 128 + 128],
                                qrt[64:128, g * S + q0: g * S + qs + 512],
                                start=True, stop=True)
                            ex = exps.tile([128, 1024], BF, tag="ex")
                            if exp3d:
                                nc.scalar.activation(
                                    ex[:].rearrange("p (t q) -> p t q", t=2)[:, :, lo:],
                                    pss[:].rearrange("p (t q) -> p t q", t=2)[:, :, lo:],
                                    AF.Exp, scale=0.125)
                            else:
                                nc.scalar.activation(ex[:, lo:512], pss[:, lo:512],
                                                     AF.Exp, scale=0.125)
                                nc.scalar.activation(ex[:, 512 + lo:1024],
                                                     pss[:, 512 + lo:1024],
                                                     AF.Exp, scale=0.125)
                            if kt >= 4 * w:                   # diagonal tile
                                for off in (0, 512):
                                    nc.vector.tensor_tensor(
                                        ex[:, off + lo: off + lo + 128],
                                        ex[:, off + lo: off + lo + 128],
                                        tri[:], ALU.mult)
                            nc.tensor.matmul(
                                psc[:, lo:512],
                                vaug[:, kt * VB + 65 * hA: kt * VB + 65 * hA + 65],
                                ex[:, lo:512],
                                start=(kt == 0), stop=(kt == ktmax))
                            nc.tensor.matmul(
                                psc[:, 512 + lo:1024],
                                vaug[:, kt * VB + 65 * hB: kt * VB + 65 * hB + 65],
                                ex[:, 512 + lo:1024],
                                start=(kt == 0), stop=(kt == ktmax))
                        # denominators -> reciprocal broadcast
                        den = dens.tile([65, 1024], BF, tag="den")
                        nc.vector.tensor_copy(den[64:65, :], psc[64:65, :])
                        psb = psS.tile([64, 1024], F32, tag="pss")
                        nc.tensor.matmul(psb[:, 0:512], ones_b[64:65, 0:64],
                                         den[64:65, 0:512], start=True, stop=True)
                        nc.tensor.matmul(psb[:, 512:1024], ones_b[64:65, 0:64],
                                         den[64:65, 512:1024], start=True, stop=True)
                        bc = bcs.tile([64, 1024], F32, tag="bc")
                        nc.vector.reciprocal(bc[:], psb[:])
                        # normalize: head A straight into ctxt, head B staged+DMA
                        nc.vector.tensor_tensor(
                            ctxt[0:64, g * S + qs: g * S + qs + 512],
                            psc[0:64, 0:512], bc[:, 0:512], ALU.mult)
                        sg = stg.tile([64, 512], BF, tag="sg")
                        nc.vector.tensor_tensor(
                            sg[:], psc[0:64, 512:1024], bc[:, 512:1024], ALU.mult)
                        nc.sync.dma_start(
                            ctxt[64:128, g * S + qs: g * S + qs + 512], sg[:])
                    # output projection for this q-chunk
                    for mt in range(DMT):
                        pso = psS.tile([128, 512], F32, tag="pss")
                        for g in range(NMT):
                            nc.tensor.matmul(
                                pso[:],
                                wo_sb[:, g * D + 128 * mt: g * D + 128 * mt + 128],
                                ctxt[:, g * S + qs: g * S + qs + 512],
                                start=(g == 0), stop=(g == NMT - 1))
                        osb = outs.tile([128, 512], BF, tag="osb")
                        nc.vector.tensor_copy(osb[:], pso[:])
                        nc.sync.dma_start(
                            out_pt[128 * mt:128 * mt + 128, qs:qs + 512], osb[:])

                def emit_proj(w):
                    proj_qk(xt_k, wk_sb, bk_sb if bias_k else None, krt, w)
                    proj_v(w)
                    if w == 0:
                        for k in range(8):
                            nc.sync.dma_start(
                                wq_sb[:, k * HD:(k + 1) * HD],
                                wqt[k * 128:(k + 1) * 128, :])
                    proj_qk(xt_q, wq_sb, bq_sb if bias_q else None, qrt, w)

                def emit_attend(w):
                    if w == 0:
                        nc.sync.dma_start(wo_sb[:], wot[:])
                    attend(w)

                if fuse:
                    for w in range(NWIN):
                        if phases[0]:
                            emit_proj(w)
                        if phases[1]:
                            emit_attend(w)
                else:
                    if phases[0]:
                        for w in range(NWIN):
                            emit_proj(w)
                    if phases[1]:
                        for w in range(NWIN):
                            emit_attend(w)

    nc.finalize()
    return nc


def host_prep(query, key, value, w_q, b_q, w_k, b_k, w_v, b_v, w_o):
    """Build the 8 per-core input maps (numpy, bf16 operands)."""
    f32 = np.float32
    inv_freq = 1.0 / (10000.0 ** (np.arange(0, DK, 2, dtype=np.float64) / DK))
    t = np.arange(S, dtype=np.float64)
    freqs = np.outer(t, inv_freq)                       # [S, 32]
    emb = np.concatenate([freqs, freqs], axis=-1)       # [S, 64]
    cos_tab = np.cos(emb).astype(f32)                   # [S, 64]
    sin_tab = np.sin(emb).astype(f32)
    cost = np.ascontiguousarray(np.tile(cos_tab.T, (2, 1))).astype(bf16)
    sint = np.ascontiguousarray(np.tile(sin_tab.T, (2, 1))).astype(bf16)
    P = np.zeros((DK, DK), f32)
    for j in range(32):
        P[j, 2 * j + 1] = -1.0
        P[32 + j, 2 * j] = 1.0
    Pblk = np.zeros((128, 128), f32)
    Pblk[:64, :64] = P
    Pblk[64:, 64:] = P
    pmat = np.ascontiguousarray(Pblk.T).astype(bf16)
    trim = np.ascontiguousarray(np.tril(np.ones((128, 128), f32)).T).astype(bf16)

    in_maps = []
    for c in range(8):
        b = c // 2
        h0 = HC * (c % 2)
        sl = slice(DK * h0, DK * h0 + HD)
        wo_slice = w_o[:, sl].T.astype(f32)             # [512, 1024]
        wot = np.ascontiguousarray(
            np.concatenate([wo_slice[128 * g:128 * (g + 1), :] for g in range(NMT)],
                           axis=1)).astype(bf16)        # [128, 4*1024]
        m = {
            "xt_q": np.ascontiguousarray(query[b].T).astype(bf16),
            "xt_k": np.ascontiguousarray(key[b].T).astype(bf16),
            "xt_v": np.ascontiguousarray(value[b].T).astype(bf16),
            "wqt": np.ascontiguousarray(w_q[sl, :].T).astype(bf16),
            "wkt": np.ascontiguousarray(w_k[sl, :].T).astype(bf16),
            "wvt": np.ascontiguousarray(w_v[sl, :].T).astype(bf16),
            "wot": wot,
            "pmat": pmat,
            "cost": cost,
            "sint": sint,
            "trim": trim,
        }
        if np.any(b_q != 0):
            m["bqp"] = np.ascontiguousarray(
                b_q[sl].reshape(NMT, 128).T).astype(f32)
        if np.any(b_k != 0):
            m["bkp"] = np.ascontiguousarray(
                b_k[sl].reshape(NMT, 128).T).astype(f32)
        if np.any(b_v != 0):
            m["bvp"] = np.ascontiguousarray(b_v[sl][None, :]).astype(bf16)
        in_maps.append(m)
    return in_maps


def assemble(results, b_o):
    out = np.zeros((B, S, D), np.float32)
    for c in range(8):
        out[c // 2] += results[c]["out_pt"].T.astype(np.float32)
    out += b_o.astype(np.float32)
    return out


_CACHE = {}


def kernel(query, key, value, mask, w_q, b_q, w_k, b_k, w_v, b_v, w_o, b_o):
    import numpy as _np
    from concourse.bass_utils import run_bass_kernel_spmd

    query = _np.asarray(query, dtype=_np.float32)
    key = _np.asarray(key, dtype=_np.float32)
    value = _np.asarray(value, dtype=_np.float32)
    w_q = _np.asarray(w_q, dtype=_np.float32)
    w_k = _np.asarray(w_k, dtype=_np.float32)
    w_v = _np.asarray(w_v, dtype=_np.float32)
    w_o = _np.asarray(w_o, dtype=_np.float32)
    b_q = _np.asarray(b_q, dtype=_np.float32)
    b_k = _np.asarray(b_k, dtype=_np.float32)
    b_v = _np.asarray(b_v, dtype=_np.float32)
    b_o = _np.asarray(b_o, dtype=_np.float32)

    variant = (bool(_np.any(b_q != 0)), bool(_np.any(b_k != 0)),
               bool(_np.any(b_v != 0)))
    if variant not in _CACHE:
        _CACHE[variant] = build_nc(with_bias=variant)
    nc = _CACHE[variant]

    in_maps = host_prep(query, key, value, w_q, b_q, w_k, b_k, w_v, b_v, w_o)
    res = run_bass_kernel_spmd(nc, in_maps, core_ids=list(range(8)))
    return assemble(res.results, b_o)
